# revision 29
# baseline (speedup 1.0000x reference)
"""Trainium2 Bass kernel for causal self-attention (B=4, S=2048, C=2048, H=16).

Sharding over 8 NeuronCores: core = 2*batch + head_group
  - data-parallel over the 4 batches (outer axis)
  - tensor-parallel over heads within a batch: 2 groups x 8 heads
Each core computes qkv projection for its head group, block-causal
flash-style attention for its 8 heads, and a partial output projection
(contraction over its 1024 w_proj rows). The host sums the two partial
outputs per batch and adds b_proj ("all-reduce" done during unshard).

Device compute is bf16 with f32 PSUM accumulation. All auxiliary PE
matmuls (row-sum reduce, reciprocal broadcast) use bf16 operands so they
run at 1 cycle/row instead of fp32's 4; the causal diag mask and the v
bias are applied on DVE instead of PE. The t=0 attention block is
interleaved into the q-projection head loop so its ACT/DVE chains hide
behind q matmuls; w_proj streams in two halves so it never coexists with
the full x slab in SBUF.
"""

from contextlib import ExitStack

import numpy as np
import ml_dtypes

import concourse.bass as bass
import concourse.bass_isa as bass_isa
import concourse.tile as tile
from concourse import bacc, mybir
from concourse.bass_utils import run_bass_kernel_spmd

BF16 = mybir.dt.bfloat16
F32 = mybir.dt.float32
ExpF = mybir.ActivationFunctionType.Exp
NPBF16 = ml_dtypes.bfloat16

B, S, C, H = 4, 2048, 2048, 16
D = 128
N_CORES = 8
NH = 8              # heads per core
NQ = NH * D         # 1024 q (=k=v) columns per core
SQT = 512           # sq tile width


def _build(compile=True, reps=1):
    CK = C // 128            # contraction chunks
    NST = S // SQT           # s tiles of 512
    NSC = S // 128           # s chunks of 128
    NB_QK = 2 * NQ // 128    # q+k output chunks of 128
    NVT = NQ // 256          # v n-tiles of 256
    ET = C // 512            # proj e tiles
    scale = 1.0 / float(np.sqrt(float(D)))

    nc = bacc.Bacc(
        "TRN2",
        target_bir_lowering=False,
        debug=False,
        enable_asserts=False,
        num_devices=N_CORES,
    )
    xT_d = nc.dram_tensor("xT", [128, NSC * CK * 128], BF16, kind="ExternalInput").ap()
    # weights preswizzled on host: per-partition-contiguous chunk runs
    wqk_d = nc.dram_tensor("wqk", [128, NB_QK * CK * 128], BF16, kind="ExternalInput").ap()
    wv_d = nc.dram_tensor("wv", [128, NVT * CK * 256], BF16, kind="ExternalInput").ap()
    wp_d = nc.dram_tensor("wp", [128, NH * C], BF16, kind="ExternalInput").ap()
    bqkvcol_d = nc.dram_tensor(
        "bqkvcol", [128, NB_QK], BF16, kind="ExternalInput"
    ).ap()
    bvb_d = nc.dram_tensor("bvb", [128, NQ], BF16, kind="ExternalInput").ap()
    utri_d = nc.dram_tensor("utri", [128, 128], BF16, kind="ExternalInput").ap()
    out_d = nc.dram_tensor("out", [S, C], BF16, kind="ExternalOutput").ap()

    with tile.TileContext(nc) as tc, ExitStack() as top:
        persist = top.enter_context(tc.tile_pool(name="persist", bufs=1))
        # q_sb/k_sb: [d, h, s]; after attention, yT_h overwrites q_sb[:, h, :]
        q_sb = persist.tile([128, NH, S], BF16, tag="q")
        k_sb = persist.tile([128, NH, S], BF16, tag="k")
        # v_sb: [s%128, s//128, h*128+d], natural v layout per s-chunk
        v_sb = persist.tile([128, NSC, NQ], BF16, tag="v")
        utri_sb = persist.tile([128, 128], BF16, tag="utri")
        # q/k bias as per-partition columns: bias_col[p, nb] = bqkv[nb*128 + p]
        bias_col = persist.tile([128, NB_QK], BF16, tag="bias_col")
        # v bias broadcast across partitions: bvb[p, n] = bqkv_v[n]
        bvb_sb = persist.tile([128, NQ], BF16, tag="bvb")
        ones_col_bf = persist.tile([128, 1], BF16, tag="ones_col_bf")
        ones_row_bf = persist.tile([1, 128], BF16, tag="ones_row_bf")

        nc.gpsimd.dma_start(out=utri_sb, in_=utri_d)
        nc.gpsimd.dma_start(out=bias_col, in_=bqkvcol_d)
        nc.vector.memset(ones_col_bf, 1.0)
        nc.vector.memset(ones_row_bf, 1.0)

        wp_src = wp_d.rearrange("p (h e) -> p h e", h=NH)
        dma_engs = [nc.sync, nc.scalar]
        out_engs = [nc.sync, nc.gpsimd]

        for _rep in range(reps):
            with ExitStack() as rep_stack:
                ph1wqk = rep_stack.enter_context(tc.tile_pool(name="ph1wqk", bufs=2))
                att = rep_stack.enter_context(tc.tile_pool(name="att", bufs=4))
                wpx = {}  # set once the wp pool opens (after xfull dies)

                def emit_proj(t_src, lo, hi, tag="po", bufs=2):
                    # et-major so early tiles only need the lo wp half
                    tiles = [
                        (sqc, et)
                        for et in range(ET)
                        for sqc in range(4 * t_src, 4 * (t_src + 1))
                    ]
                    for sqc, et in tiles[lo:hi]:
                        wp_pool, wp = wpx["wp"]
                        ps_o = ps2.tile([128, 512], F32, tag=tag, bufs=bufs)
                        for hp in range(NH):
                            nc.tensor.matmul(
                                ps_o,
                                lhsT=q_sb[:, hp, bass.ts(sqc, 128)],
                                rhs=wp[:, hp, bass.ts(et, 512)],
                                start=(hp == 0),
                                stop=(hp == NH - 1),
                            )
                        o_sb = wp_pool.tile([128, 512], BF16, tag="o", bufs=3)
                        nc.scalar.copy(o_sb, ps_o)
                        out_engs[(sqc * ET + et) % 2].dma_start(
                            out=out_d[bass.ts(sqc, 128), bass.ts(et, 512)],
                            in_=o_sb,
                        )

                pending = None   # previous head awaiting normalization

                def flush_pending():
                    nonlocal pending
                    if pending is None:
                        return
                    yu_p, rs_p, h_p, tsl_p = pending
                    # yT (bf16) overwrites q_sb[:, h_p, tsl_p]
                    nc.vector.tensor_mul(q_sb[:, h_p, tsl_p], yu_p, rs_p)
                    pending = None

                def attend(t, h):
                    nonlocal pending
                    tsl = bass.ts(t, SQT)
                    nsk = 4 * t + 4  # block-causal sk chunks
                    ahead = 2
                    ps_yu = ps2.tile([128, 512], F32, tag="yu", bufs=2)
                    # row-sum partials accumulate on DVE into a single acc
                    acc = att.tile([128, 512], BF16, tag="acc", bufs=2, name="acc")
                    sc_tiles = {}

                    def emit_scores(j):
                        off = 0 if j < 4 * t else (j - 4 * t) * 128
                        w = 512 - off
                        ps_sc = ps2.tile([128, 512], F32, tag="sc", bufs=4)
                        # scoresT[sk, sq] = k_h.T q_h (live sq cols only)
                        nc.tensor.matmul(
                            ps_sc[:, :w],
                            lhsT=k_sb[:, h, bass.ts(j, 128)],
                            rhs=q_sb[:, h, t * SQT + off : (t + 1) * SQT],
                            start=True,
                            stop=True,
                        )
                        sc_tiles[j] = (ps_sc, off, w)

                    # Process the narrow diagonal chunks early (right after
                    # two full-width ones that initialize the row-sum accs)
                    # so the head's tail is all 512-wide, PE-balanced work.
                    if t == 0:
                        order = list(range(nsk))
                    else:
                        diag = list(range(4 * t, 4 * t + 4))
                        rest = [j for j in range(2, 4 * t)]
                        order = [0, 1] + diag + rest
                    for j in order[:min(ahead, nsk)]:
                        emit_scores(j)
                    for idx, j in enumerate(order):
                        ps_sc, off, w = sc_tiles.pop(j)
                        e = att.tile([128, 512], BF16, tag="e", bufs=6)
                        nc.scalar.activation(
                            out=e[:, off:], in_=ps_sc[:, :w], func=ExpF,
                            scale=scale,
                        )
                        if j >= 4 * t:
                            # causal mask for the diagonal 128-block on DVE
                            nc.vector.tensor_mul(
                                e[:, off : off + 128],
                                e[:, off : off + 128],
                                utri_sb,
                            )
                        if idx + ahead < nsk:
                            emit_scores(order[idx + ahead])
                        if idx == 0:
                            flush_pending()
                        # row sums; the first touch is full width
                        if idx == 0:
                            nc.vector.tensor_copy(acc, e)
                        else:
                            nc.vector.tensor_add(
                                acc[:, off:], acc[:, off:], e[:, off:]
                            )
                        # yu[d, sq] += v[sk, d].T @ e[sk, sq] (live region)
                        nc.tensor.matmul(
                            ps_yu[:, off:],
                            lhsT=v_sb[:, j, bass.ts(h, 128)],
                            rhs=e[:, off:],
                            start=(idx == 0),
                            stop=(idx == nsk - 1),
                        )
                    # reduce+broadcast the exp sums across partitions on the
                    # idle GpSimd engine, then reciprocal on DVE
                    rsall = att.tile([128, 512], F32, tag="rsall", bufs=2)
                    nc.gpsimd.partition_all_reduce(
                        rsall, acc, 128, bass_isa.ReduceOp.add
                    )
                    rs_sb = att.tile([128, 512], BF16, tag="rsb", bufs=2)
                    with nc.allow_low_precision(reason="bf16 1/rowsum"):
                        nc.vector.reciprocal(rs_sb, rsall)
                    # interleave prev t-block's projection tiles: fills PE
                    # while this head's reciprocal completes on DVE
                    if t > 0:
                        emit_proj(t - 1, 2 * h, 2 * h + 2)
                    pending = (ps_yu, rs_sb, h, tsl)

                ph1x_cm = tc.tile_pool(name="ph1x", bufs=1)
                ph1x = ph1x_cm.__enter__()
                if True:
                    # ------------- Phase 1: QKV projection -------------
                    # x fully resident; each weight column chunk read once.
                    # Section order: v, then k, then q; t=0 attention is
                    # interleaved per-head into the q section.
                    xfull = ph1x.tile([128, CK, S], BF16, tag="xf")
                    for sc in range(NSC):
                        # slab sc: per-partition contiguous 4KB run of
                        # host-swizzled xT; slab 0 in ck-quarters so the
                        # very first matmul chain unblocks asap
                        src = xT_d[
                            :, sc * CK * 128 : (sc + 1) * CK * 128
                        ].rearrange("p (ck sl) -> p ck sl", ck=CK)
                        pieces = 4 if sc == 0 else 1
                        step = CK // pieces
                        for i in range(pieces):
                            sl = slice(i * step, (i + 1) * step)
                            dma_engs[sc % 2].dma_start(
                                out=xfull[:, sl, bass.ts(sc, 128)],
                                in_=src[:, sl, :],
                            )

                    def emit_qk(sec, hh, pspool, pstag, psbufs):
                        nb = sec * NH + hh
                        wt = ph1wqk.tile(
                            [128, CK, 128], BF16, tag="wqk", name="wqk", bufs=2
                        )
                        dma_engs[nb % 2].dma_start(
                            out=wt,
                            in_=wqk_d[
                                :, nb * CK * 128 : (nb + 1) * CK * 128
                            ].rearrange("p (ck n) -> p ck n", ck=CK),
                        )
                        dest = q_sb if sec == 0 else k_sb
                        for st in range(NST):
                            ps = pspool.tile(
                                [128, 512], F32, tag=pstag, bufs=psbufs
                            )
                            for ck in range(CK):
                                nc.tensor.matmul(
                                    ps,
                                    lhsT=wt[:, ck, :],
                                    rhs=xfull[:, ck, bass.ts(st, 512)],
                                    start=(ck == 0),
                                    stop=(ck == CK - 1),
                                )
                            # copy + per-partition bias add on ScalarE
                            nc.scalar.add(
                                dest[:, hh, bass.ts(st, 512)],
                                ps,
                                bias_col[:, nb : nb + 1],
                            )

                    with tc.tile_pool(name="ps1", bufs=4, space="PSUM") as ps1:
                        with tc.tile_pool(name="ph1wv", bufs=2) as ph1wv:

                            def load_wv(nt, pieces=2):
                                wt = ph1wv.tile([128, CK, 256], BF16, tag="wv")
                                # split along ck so matmuls unblock earlier
                                src = wv_d[
                                    :, nt * CK * 256 : (nt + 1) * CK * 256
                                ].rearrange("p (ck n) -> p ck n", ck=CK)
                                step = CK // pieces
                                for i in range(pieces):
                                    sl = slice(i * step, (i + 1) * step)
                                    nc.gpsimd.dma_start(
                                        out=wt[:, sl, :], in_=src[:, sl, :]
                                    )
                                return wt

                            # v weights on the gpsimd queue (x owns sync/
                            # scalar); wv0 in quarters so the first matmul
                            # unblocks at ~128KB of DMA
                            wt0 = load_wv(0, pieces=4)
                            nc.gpsimd.dma_start(out=bvb_sb, in_=bvb_d)
                            # v: n-tiles of 256, psum[s 128, n 256]
                            for nt in range(NVT):
                                wt = wt0 if nt == 0 else load_wv(nt)
                                for sc in range(NSC):
                                    ps = ps1.tile([128, 512], F32, tag="psv", bufs=4)
                                    psv = ps[:, :256]
                                    for ck in range(CK):
                                        nc.tensor.matmul(
                                            psv,
                                            lhsT=xfull[:, ck, bass.ts(sc, 128)],
                                            rhs=wt[:, ck, :],
                                            start=(ck == 0),
                                            stop=(ck == CK - 1),
                                        )
                                    # copy + broadcast bias add on DVE
                                    nc.vector.tensor_add(
                                        v_sb[:, sc, bass.ts(nt, 256)],
                                        psv,
                                        bvb_sb[:, bass.ts(nt, 256)],
                                    )
                        # k, transposed: psum[n 128, s 512]
                        for hh in range(NH):
                            emit_qk(1, hh, ps1, "psqk", 4)

                    # q section + t=0 attention share the ps2 "sc" ring:
                    # sc(3) + yu(2) + rs(1) + bc(1) + po(1) = 8 banks
                    ps2 = rep_stack.enter_context(
                        tc.tile_pool(name="ps2", bufs=1, space="PSUM")
                    )
                    for hh in range(NH):
                        emit_qk(0, hh, ps2, "po", 2)
                        if hh < NH - 1:
                            attend(0, hh)
                # xfull freed; stream in w_proj (low-et columns first so
                # the first proj tiles unblock earliest); the last t=0 head
                # attends after the wp DMA is already in flight
                ph1x_cm.__exit__(None, None, None)
                wppool = rep_stack.enter_context(tc.tile_pool(name="wpp", bufs=1))
                wp = wppool.tile([128, NH, C], BF16, tag="wp", bufs=1)
                for half in (0, 1):
                    lo = half * (C // 2)
                    nc.sync.dma_start(
                        out=wp[:, : NH // 2, lo : lo + C // 2],
                        in_=wp_src[:, : NH // 2, lo : lo + C // 2],
                    )
                    nc.gpsimd.dma_start(
                        out=wp[:, NH // 2 :, lo : lo + C // 2],
                        in_=wp_src[:, NH // 2 :, lo : lo + C // 2],
                    )
                wpx["wp"] = (wppool, wp)
                attend(0, NH - 1)
                for t in range(1, NST):
                    for h in range(NH):
                        attend(t, h)
                flush_pending()
                emit_proj(NST - 1, 0, 4 * ET)

    if compile:
        nc.compile()
    return nc


def _make_utri():
    """utri[p, f] = 1 if p <= f else 0 (keep sk<=sq within diag block)."""
    return np.triu(np.ones((128, 128), np.float32)).astype(NPBF16)


_NC_CACHE = None


def _get_nc():
    global _NC_CACHE
    if _NC_CACHE is None:
        _NC_CACHE = _build()
    return _NC_CACHE


def _make_in_maps(x, w_qkv, b_qkv, w_proj):
    utri = _make_utri()
    CK = C // 128
    in_maps = []
    for core in range(N_CORES):
        b = core // 2
        g = core % 2
        cs = slice(g * NQ, (g + 1) * NQ)
        xb = np.asarray(x[b], np.float32).astype(NPBF16)
        # xh[p, sc, ck, sl] = x[sc*128+sl, ck*128+p], flattened to [128, S*C/128]
        xT = np.ascontiguousarray(
            xb.reshape(S // 128, 128, C // 128, 128).transpose(3, 0, 2, 1)
        ).reshape(128, (S // 128) * (C // 128) * 128)
        wqkv_c = np.concatenate(
            [w_qkv[:, cs], w_qkv[:, C:][:, cs], w_qkv[:, 2 * C:][:, cs]], axis=1
        ).astype(NPBF16)
        # q/k weights: [p, nb, ck, n] with 128-col chunks
        wqk = np.ascontiguousarray(
            wqkv_c[:, : 2 * NQ]
            .reshape(CK, 128, 2 * NQ // 128, 128)
            .transpose(1, 2, 0, 3)
        ).reshape(128, -1)
        # v weights: [p, nt, ck, n] with 256-col chunks
        wv = np.ascontiguousarray(
            wqkv_c[:, 2 * NQ:]
            .reshape(CK, 128, NQ // 256, 256)
            .transpose(1, 2, 0, 3)
        ).reshape(128, -1)
        bqkv_c = np.concatenate(
            [b_qkv[cs], b_qkv[C:][cs], b_qkv[2 * C:][cs]]
        ).astype(NPBF16)
        bqkvcol = np.ascontiguousarray(
            bqkv_c[: 2 * NQ].reshape(2 * NQ // 128, 128).T
        )
        bvb = np.ascontiguousarray(
            np.broadcast_to(bqkv_c[2 * NQ:], (128, NQ))
        )
        # proj weights: [p, h, e]
        wp = np.ascontiguousarray(
            np.asarray(w_proj[cs, :], np.float32)
            .astype(NPBF16)
            .reshape(NH, 128, C)
            .transpose(1, 0, 2)
        ).reshape(128, -1)
        in_maps.append(
            {
                "xT": xT,
                "wqk": wqk,
                "wv": wv,
                "wp": wp,
                "bqkvcol": bqkvcol,
                "bvb": bvb,
                "utri": utri,
            }
        )
    return in_maps


def kernel(x, w_qkv, b_qkv, w_proj, b_proj):
    x = np.asarray(x, np.float32)
    w_qkv = np.asarray(w_qkv, np.float32)
    b_qkv = np.asarray(b_qkv, np.float32)
    w_proj = np.asarray(w_proj, np.float32)
    b_proj = np.asarray(b_proj, np.float32)

    nc = _get_nc()
    in_maps = _make_in_maps(x, w_qkv, b_qkv, w_proj)
    res = run_bass_kernel_spmd(nc, in_maps, core_ids=list(range(N_CORES)))

    out = np.empty((B, S, C), np.float32)
    for b in range(B):
        out[b] = np.asarray(res.results[2 * b]["out"], np.float32)
        out[b] += np.asarray(res.results[2 * b + 1]["out"], np.float32)
        out[b] += b_proj[None, :]
    return out
